# revision 1
# baseline (speedup 1.0000x reference)
"""Trainium2 Bass kernel V2 for the PixelRNN Diagonal BiLSTM problem.

Contract: kernel(**inputs) takes FULL unsharded inputs and returns the FULL
(32, 3, 256, 32, 32) float32 output. Pure data-parallel over 8 NeuronCores
(4 images each), weights replicated, no collectives.

V2 vs V1 (see layout_sim2.py for formula validation; 350 us vs 681 us):
  * All matmul operands bf16: fused ldweights is 1 cyc/row and streams
    1 cyc/col at ANY moving size (f32r pays 4 cyc/row weight loads and
    4 cyc/col under 256 moving cols).
  * Xr: a reversed-coordinate copy of the in-projected image, so the bwd
    scan reads single-AP windows like fwd (V1 used 3 strided pieces per
    matmul -> 3x weight loads).
  * H buffers eliminated: h(t) lands in a per-step contiguous hstage slice
    (fresh slice per step -> no WAR on rotation); the next step's Wh reads
    it directly (window shifts by one diagonal = 4 cols), and 2 small DVE
    ops per step-dir scatter the finished h values straight into `un`
    (replaces V1's whole gather phase AND the strided 16-bit h writes).
  * Layout col = 128d + 32b + s kept: the {128,32} stride set keeps
    same-engine APs provably disjoint mod 32 for the sync framework
    (every instruction must carry <= 1 sem wait).
  * in_proj psum is staged to SBUF per engine (ACT: pin0, DVE: pin1) and
    both engines scatter X / Xr in parallel (cross-engine reads of one
    psum tile serialize).
  * out_proj evicts alternate ACT/DVE by psum group onto the dead X / Xr
    buffers; bf16 output, 6 paired DMAs on fresh SWDGE lanes.

Per-core SBUF: X/Xr [128, 12288] bf16, hstage [128, 64, 384] bf16,
C [128, 2, 384] f32, un [128, 8192] bf16, weights [128, 3584] bf16.
"""
from contextlib import ExitStack

import numpy as np

import concourse.bass as bass
import concourse.tile as tile
from concourse.tile import add_dep_helper
from concourse import mybir
from concourse.bass_utils import run_bass_kernel_spmd

AF = mybir.ActivationFunctionType
F32 = mybir.dt.float32
BF16 = mybir.dt.bfloat16
U32 = mybir.dt.uint32

BS = 4            # batch shard per core
NCORES = 8
H = W = 32
HC = 128
D = 95            # diag blocks incl always-pad d=94
S = 32
NCOL = D * 128
NCOLX = 96 * 128  # X/Xr padded to 24 eviction chunks (12288 bf16 cols)


def _ap(t, off, dims):
    """Free-dim-strided AP on SBUF tile t: dims = [[stride, count], ...]."""
    a = t[:, :]
    return bass.AP(tensor=a.tensor, offset=a.offset + off, ap=[a.ap[0]] + dims)


def build(nc):
    # ---------------- DRAM I/O ----------------
    xT_d = nc.dram_tensor("xT", (3, BS * 1024), BF16, kind="ExternalInput")
    ipw_d = nc.dram_tensor("in_projT", (3, HC), BF16, kind="ExternalInput")
    wpack_d = nc.dram_tensor("wpack", (HC, 3584), BF16, kind="ExternalInput")
    vpack_d = nc.dram_tensor("vpack", (HC, 15), F32, kind="ExternalInput")
    out_d = nc.dram_tensor("out", (HC, 6, BS * 1024), BF16,
                           kind="ExternalOutput")

    with tile.TileContext(nc) as tc, ExitStack() as ctx:
        const = ctx.enter_context(tc.tile_pool(name="const", bufs=1))
        big = ctx.enter_context(tc.tile_pool(name="big", bufs=1))
        hpool = ctx.enter_context(tc.tile_pool(name="hpool", bufs=1))
        # bufs=3: a step-(t) tile reuses the step-(t-2) other-direction
        # buffer, whose readers' DVE ticks are already covered by the
        # preceding tc op's wait -> sigma keeps a single (PE) sync wait
        etmp = ctx.enter_context(tc.tile_pool(name="etmp", bufs=4))
        ev = ctx.enter_context(tc.tile_pool(name="ev", bufs=1))
        psum = ctx.enter_context(tc.tile_pool(name="psum", bufs=1,
                                              space="PSUM"))

        final_insts = []
        ipw = const.tile([3, HC], BF16)
        final_insts.append(nc.sync.dma_start(ipw, ipw_d.ap()))
        xT = const.tile([3, BS * 1024], BF16)
        final_insts.append(nc.sync.dma_start(xT, xT_d.ap()))
        wpk = const.tile([HC, 3584], BF16)
        final_insts.append(nc.sync.dma_start(wpk, wpack_d.ap()))
        vpk = const.tile([HC, 15], F32)
        final_insts.append(nc.sync.dma_start(vpk, vpack_d.ap()))

        wi = {'f': wpk[:, 0:512], 'b': wpk[:, 1024:1536]}
        wh = {'f': wpk[:, 512:1024], 'b': wpk[:, 1536:2048]}
        owf = wpk[:, 2048:2816]
        owb = wpk[:, 2816:3584]
        bias = {'f': vpk[:, 0:4], 'b': vpk[:, 4:8]}      # per-gate biases
        ob = vpk[:, 8:14]
        ipb = vpk[:, 14:15]

        X = big.tile([HC, NCOLX], BF16, tag="X")
        Xr = big.tile([HC, NCOLX], BF16, tag="Xr")
        un = big.tile([HC, 2 * BS * 1024], BF16, tag="un")
        C = const.tile([HC, 2, 4 * 96], F32, name="C")
        # per-step h staging: one fresh slice per (t, dir) -> no buffer
        # reuse, hence no DVE-waits-PE WAR on rotation (49 KiB/partition)
        hstall = const.tile([HC, 2 * S, 4 * 96], BF16, name="hstall")

        nc.gpsimd.memset(X.bitcast(U32), 0)
        final_insts.append(nc.gpsimd.memset(Xr.bitcast(U32), 0))

        # ---- per-engine pre-observers: each engine consumes the DMA /
        # memset semaphores via tiny single-wait ops so real instructions
        # keep <=1 sync wait (matmuls have a single HW wait slot).  All PE
        # observers are garbage ldweights (legal for bf16, no psum writes).
        hb_nc = Xr[:, NCOLX - 1:NCOLX]        # (d=95 pad, never used)
        nc.tensor.ldweights(ipw[:, 0:1])
        nc.tensor.ldweights(xT[:, 0:1])
        trash_a = const.tile([HC, 4], F32)
        trash_d = const.tile([HC, 24], F32)
        nc.scalar.activation(trash_a[:, 0:1], vpk[:, 0:1], AF.Copy)  # vpack
        nc.scalar.activation(trash_a[:, 1:2], hb_nc, AF.Copy)        # memsets
        nc.vector.tensor_copy(trash_d[:, 0:1], vpk[:, 0:1])
        nc.vector.tensor_copy(trash_d[:, 1:2], hb_nc)
        tc.no_sync_barrier()

        # ---------------- phase 1: in_proj + scatter ----------------
        with nc.named_scope("in_proj"):
            # pin psum col = 512b + 32(r%16) + j, tile rh = r//16
            pin = {0: psum.tile([HC, 4, 512], F32, tag="Pf", name="pin0"),
                   1: psum.tile([HC, 4, 512], F32, tag="Pb", name="pin1")}
            for b in range(BS):
                for rh in range(2):
                    c0 = (b * 2 + rh) * 512
                    nc.tensor.matmul(pin[rh][:, b, :], ipw,
                                     xT[:, c0:c0 + 512], start=True, stop=True)
            # late consts: observe only after in_proj mms are queued so the
            # PE can start projecting while wpack is still in flight.
            nc.tensor.ldweights(wpk[:, 0:1])                  # wpack DMA
            nc.tensor.ldweights(hb_nc)                        # memsets
            # Cross-engine reads of one psum tile serialize (the framework
            # chains the second engine behind the first's last read), so
            # each pin tile has ONE reader engine: ACT evicts pin0 -> st0,
            # DVE evicts pin1 -> st1 (bias folded in here), and both
            # engines then scatter from SBUF staging in parallel.
            st = {0: const.tile([HC, 2048], BF16, name="st0"),
                  1: const.tile([HC, 2048], BF16, name="st1")}
            nc.scalar.activation(st[0][:, :],
                                 pin[0][:, :, :].rearrange("p a b -> p (a b)"),
                                 AF.Identity, bias=ipb)
            nc.vector.tensor_scalar_add(
                st[1][:, :], pin[1][:, :, :].rearrange("p a b -> p (a b)"),
                ipb)

            # ---- X scatter on ACT (case A all rows + A-ext + case B) ----
            # A: col = 257r + 128j + 32b (valid d<=62; rows>=16 extended,
            # the garbage provably lands on case-B pixel cells, fixed below)
            src = _ap(st[0], 0, [[512, BS], [32, 16], [1, 32]])
            dst = _ap(X, 0, [[32, BS], [257, 16], [128, 32]])
            nc.scalar.activation(dst, src, AF.Copy)
            # ACT observes the DVE st1-evict tick before reading st1
            nc.scalar.activation(trash_a[0:1, 3:4], st[1][0:1, 0:1], AF.Copy)
            src = _ap(st[1], 0, [[512, BS], [32, 16], [1, 32]])
            dst = _ap(X, 257 * 16, [[32, BS], [257, 16], [128, 32]])
            nc.scalar.activation(dst, src, AF.Copy)
            for r in range(16, 32):
                na = 63 - 2 * r
                nb = 32 - na
                srcB = _ap(st[1], 32 * (r - 16) + na, [[512, BS], [1, nb]])
                dstB = _ap(X, 255 * r + 127 * na + 62, [[32, BS], [127, nb]])
                last_x_op = nc.scalar.activation(dstB, srcB, AF.Copy)

            # ---- Xr scatter on DVE (exact A' rows 0-15, B', big B') ----
            for r in range(16):
                na = 32 - 2 * r
                srcA = _ap(st[0], 32 * r, [[512, BS], [1, na]])
                dstA = _ap(Xr, 257 * r, [[32, BS], [129, na]])
                last_xr_op = nc.vector.tensor_copy(dstA, srcA)
                nb = 32 - na
                if nb:
                    srcB = _ap(st[0], 32 * r + na, [[512, BS], [1, nb]])
                    dstB = _ap(Xr, 255 * r + 128 * na + 31,
                               [[32, BS], [128, nb]])
                    last_xr_op = nc.vector.tensor_copy(dstB, srcB)
            srcBB = _ap(st[1], 0, [[512, BS], [32, 16], [1, 32]])
            dstBB = _ap(Xr, 255 * 16 + 31, [[32, BS], [255, 16], [128, 32]])
            last_xr_op = nc.vector.tensor_copy(dstBB, srcBB)

        # ---------------- phase 2: scan (+ fused un scatter) ------------
        Xbuf = {'f': X, 'b': Xr}
        with nc.named_scope("scan"):
            # PE pre-observes scatter completion (frees pin psum tags AND
            # guarantees X/Xr contents) via two garbage ldweights.
            ldx = nc.tensor.ldweights(X[:, 8094:8095])        # ACT tick
            ldxr = nc.tensor.ldweights(Xr[:, 4111:4112])      # DVE tick
            first_mm_deps = [ldx, ldxr]
            tc_prev = {}
            sfio_prev = {}
            hst_prev = {}
            for t in range(S):
                nd = 94 - 2 * t
                N = BS * nd
                Nr = N
                step = {}
                for di, dr in enumerate('fb'):
                    P = psum.tile([HC, 4, 512], F32, tag='P' + dr,
                                  name=f"P{dr}{t}")
                    xap = _ap(Xbuf[dr], 129 * t, [[128, nd], [32, BS]])
                    if t >= 1:
                        # PE pre-observes the ACT tick that freed this dir's
                        # psum banks: sigma_o of t-1 is the LAST psum reader
                        # in the consumer-ordered ACT stream -- waiting on tc
                        # (which sits behind the whole DVE c-chain) would
                        # stall the Wi matmuls ~1us longer than needed.
                        ldw = nc.tensor.ldweights(sfio_prev[dr][0:1, 2, 0:1])
                        deps0 = [ldw]
                    else:
                        deps0 = first_mm_deps
                    for g in range(4):
                        mmi = nc.tensor.matmul(
                            P[:, g, 0:N], wi[dr][:, g * HC:(g + 1) * HC],
                            xap, start=True, stop=(t == 0))
                        if g == 0:
                            for dep in deps0:
                                add_dep_helper(mmi.ins, dep.ins, sync=False,
                                               reason="pre-observed")
                    if t >= 1:
                        # previous step's h, one diagonal in (contiguous)
                        hap = hst_prev[dr][:, 4:4 + N]
                        for g in range(4):
                            nc.tensor.matmul(P[:, g, 0:N],
                                             wh[dr][:, g * HC:(g + 1) * HC],
                                             hap, start=False, stop=True)
                    # ---- activations, emitted in CONSUMER order: cm
                    # needs sigma_f first, pm needs sigma_i + tg next, and
                    # hm's sigma_o is not needed until after tanh-c ----
                    sfio = etmp.tile([HC, 3, Nr], BF16, tag="sfio",
                                     name=f"sfio{dr}{t}")
                    sfio_prev[dr] = sfio
                    tgt = etmp.tile([HC, Nr], BF16, tag="tg",
                                    name=f"tg{dr}{t}")
                    for g in (1, 0, 3, 2):
                        if g == 3:
                            nc.scalar.activation(tgt, P[:, 3, 0:Nr], AF.Tanh,
                                                 bias=bias[dr][:, 3:4])
                        else:
                            nc.scalar.activation(sfio[:, g, :], P[:, g, 0:Nr],
                                                 AF.Sigmoid,
                                                 bias=bias[dr][:, g:g + 1])
                    # ---- cell update ----
                    c_sl = C[:, di, 4 * t:4 * t + Nr]
                    if t == 0:
                        nc.vector.tensor_mul(c_sl, sfio[:, 0, :], tgt)
                    else:
                        if dr == 'f':
                            # DVE pre-observes its own LAST op of step t-1
                            # (the final un-scatter) so every same-engine
                            # self-wait this step is already covered and
                            # each real op keeps a single cross-engine wait.
                            tch = nc.vector.tensor_copy(
                                trash_d[0:1, 4:5], last_un_cell)
                        cm = nc.vector.tensor_mul(c_sl, c_sl, sfio[:, 1, :])
                        if dr == 'f':
                            add_dep_helper(cm.ins, tch.ins, sync=False,
                                           reason="c touch first")
                        nc.vector.tensor_mul(tgt, sfio[:, 0, :], tgt)
                        nc.vector.tensor_add(c_sl, c_sl, tgt)
                    step[dr] = (sfio, c_sl)
                for di, dr in enumerate('fb'):
                    sfio, c_sl = step[dr]
                    tct = etmp.tile([HC, Nr], BF16, tag="tc",
                                    name=f"tc{dr}{t}")
                    act_i = nc.scalar.activation(tct, c_sl, AF.Tanh)
                    tc_prev[dr] = tct
                    hst = hstall[:, 2 * t + di, :]
                    nc.vector.tensor_mul(hst[:, 0:N], sfio[:, 2, :], tct)
                    hst_prev[dr] = hst
                    # scatter this step's finished h values into un
                    if dr == 'f':
                        pieces = [(2 * t, min(62, 2 * t + 31), 1,
                                   30 * t, 0),
                                  (max(63, 93 - 2 * t), 93 - t, 31,
                                   30 * t - 1860, 0)]
                    else:
                        pieces = [(max(t, 2 * t - 31), min(31, 2 * t), 31,
                                   -30 * t, 4096),
                                  (max(32, 62 - 2 * t), 93 - 2 * t, 1,
                                   930 - 30 * t, 4096)]
                    for dlo, dhi, dstride, base, off in pieces:
                        dlo = max(dlo, t)
                        if dhi < dlo:
                            continue
                        cnt = dhi - dlo + 1
                        col0 = off + base + dstride * dlo
                        hsrc = bass.AP(
                            tensor=hst.tensor,
                            offset=hst.offset + 4 * (dlo - t),
                            ap=[hst.ap[0]] + [[1, BS], [4, cnt]])
                        last_un = nc.vector.tensor_copy(
                            _ap(un, col0, [[1024, BS], [dstride, cnt]]),
                            hsrc)
                        last_un_cell = un[0:1, col0:col0 + 1]

        # ---------------- phase 3: output projection ----------------
        # Evictions reuse dead X / Xr (23 chunks of 512 bf16 cols each) +
        # a 2-chunk tail tile; ACT and DVE alternate chunks.
        with nc.named_scope("out_proj"):
            # read the cell written by the LAST gather op (t=31 bwd r=0
            # case B'), so the wait covers every un writer
            ldun = nc.tensor.ldweights(un[:, 4127:4128])  # un ready (DVE)
            # ACT tick of tc(31, b) covers the scan's last psum reads for
            # BOTH tags (ACT sem is monotonic)
            ldtg = nc.tensor.ldweights(tc_prev['b'][:, 0:1])
            evA = big.tile([HC, NCOLX], BF16, tag="X", name="evA")
            evB = big.tile([HC, NCOLX], BF16, tag="Xr", name="evB")

            # ACT evicts land only on X's memory (ACT-written earlier, so
            # the WAW stays same-engine); DVE evicts only on Xr's.
            def chunk_dst(q):
                half = (q // 4) % 2
                k = (q // 8) * 4 + q % 4       # per-engine chunk index 0..23
                return (evA if half == 0 else evB)[:, 512 * k:512 * (k + 1)]

            # one reader ENGINE per psum group (cross-engine reads of a psum
            # tile serialize): tag Pf groups evict on ACT, Pb groups on DVE
            last_ev = {0: None, 1: None}   # per-tag last evict dst
            for m in range(6):
                for half in range(2):
                    P = psum.tile([HC, 4, 512], F32, tag='P' + 'fb'[half],
                                  name=f"Po{m}{half}")
                    deps = []
                    if m == 0 and half == 0:
                        deps = [ldun, ldtg]
                    elif m == 0 and half == 1:
                        deps = [nc.tensor.ldweights(tc_prev['b'][:, 0:1])]
                    elif last_ev[half] is not None:
                        # PE pre-observes the evict tick that freed this tag
                        deps.append(nc.tensor.ldweights(last_ev[half]))
                        last_ev[half] = None
                    for cb in range(4):
                        ch = half * 4 + cb
                        mmi = nc.tensor.matmul(
                            P[:, cb, :], owf[:, m * HC:(m + 1) * HC],
                            un[:, ch * 512:(ch + 1) * 512],
                            start=True, stop=False)
                        if cb == 0:
                            for dep in deps:
                                add_dep_helper(mmi.ins, dep.ins, sync=False,
                                               reason="pre-observed")
                        last_mm = nc.tensor.matmul(
                            P[:, cb, :], owb[:, m * HC:(m + 1) * HC],
                            un[:, 4096 + ch * 512:4096 + (ch + 1) * 512],
                            start=False, stop=True)
                    for cb in range(4):
                        q = m * 8 + half * 4 + cb
                        dst = chunk_dst(q)
                        if half == 0:
                            last_act_evi = nc.scalar.activation(
                                dst, P[:, cb, :], AF.Identity,
                                bias=ob[:, m:m + 1])
                        else:
                            last_dve_evi = nc.vector.tensor_scalar_add(
                                dst, P[:, cb, :], ob[:, m:m + 1])
                        last_ev[half] = dst[:, 0:1]
                # DMA out per m-PAIR (6 DMAs total -> each gets a fresh
                # SWDGE lane, so no queue-FIFO wait atop the data wait)
                if m % 2 == 1:
                    k = m // 2
                    for half_, srcs in ((0, evA), (1, evB)):
                        final_insts.append(nc.gpsimd.dma_start(
                            out_d.ap()[:, 2 * k:2 * k + 2,
                                       2048 * half_:2048 * half_ + 2048],
                            _ap(srcs, 4096 * k, [[2048, 2], [1, 2048]])))
            final_insts += [last_mm, last_act_evi, last_dve_evi]
            for fi in final_insts:
                nop = nc.sync.nop()
                add_dep_helper(nop.ins, fi.ins, sync=True,
                               reason="drain diet: pre-observe final ticks")
    return nc


def _prep_inputs(inputs):
    """Host-side weight reshaping -> per-core in_maps."""
    import ml_dtypes
    bf = ml_dtypes.bfloat16

    def cast(a):
        return np.ascontiguousarray(a, np.float32).astype(bf)

    x = np.asarray(inputs['x'], np.float32)
    wpack = np.concatenate([
        np.asarray(inputs['fwd_Wi'], np.float32).T,
        np.asarray(inputs['fwd_Wh'], np.float32).T,
        np.asarray(inputs['bwd_Wi'], np.float32).T,
        np.asarray(inputs['bwd_Wh'], np.float32).T,
        np.asarray(inputs['out_w'], np.float32)[:, :HC].T,
        np.asarray(inputs['out_w'], np.float32)[:, HC:].T,
    ], axis=1)                                             # (128, 3584)
    vpack = np.concatenate([
        np.asarray(inputs['fwd_b'], np.float32).reshape(4, HC).T,
        np.asarray(inputs['bwd_b'], np.float32).reshape(4, HC).T,
        np.asarray(inputs['out_b'], np.float32).reshape(6, HC).T,
        np.asarray(inputs['in_proj_b'], np.float32).reshape(HC, 1),
    ], axis=1)                                             # (128, 15)
    common = {
        "in_projT": cast(np.asarray(inputs['in_proj_w'], np.float32).T
                         / 255.0),
        "wpack": cast(wpack),
        "vpack": np.ascontiguousarray(vpack),
    }
    in_maps = []
    for c in range(NCORES):
        xs = x[c * BS:(c + 1) * BS]
        xTc = np.ascontiguousarray(
            xs.transpose(1, 0, 2, 3).reshape(3, BS * 1024))
        in_maps.append({"xT": cast(xTc), **common})
    return in_maps


def _assemble(results):
    outs = []
    for r in results:
        lg = np.asarray(r["out"], dtype=np.float32)        # (128, 6, 4096)
        lg = lg.transpose(1, 0, 2).reshape(6, HC, BS, H, W)
        lg = lg.transpose(2, 0, 1, 3, 4)
        outs.append(lg.reshape(BS, 768, H, W))
    full = np.concatenate(outs, axis=0)
    return np.ascontiguousarray(
        full.reshape(32, 3, 256, H, W).astype(np.float32))


def kernel(**inputs):
    nc = bass.Bass("TRN2", target_bir_lowering=False, debug=False)
    build(nc)
    in_maps = _prep_inputs(inputs)
    res = run_bass_kernel_spmd(nc, in_maps, core_ids=list(range(NCORES)))
    return _assemble(res.results)


if __name__ == "__main__":
    nc = bass.Bass("TRN2", target_bir_lowering=False, debug=False)
    build(nc)
    print("IR build OK")



# revision 7
# speedup vs baseline: 1.5699x; 1.5699x over previous
"""Trainium2 Bass kernel V3 for the PixelRNN Diagonal BiLSTM problem.

Contract: kernel(**inputs) takes FULL unsharded inputs and returns the FULL
(32, 3, 256, 32, 32) float32 output. Pure data-parallel over 8 NeuronCores
(4 images each), weights replicated, no collectives.

V3 vs V2 (see sim_v3.py for geometry validation):
  * Halved scan: only ~half the skewed-grid cells are real pixels. Prefix
    pad-cell LSTM states are input-independent (x=0 there), so the host
    precomputes the 17 possible pad states from the weights and each
    diagonal's scan starts at its first real cell with a per-diagonal init
    (h_init, c_init) table. Real segments are <= 16 steps, so the scan is
    16 macro-steps/dir over 4096 columns (vs 32 steps over 8064).
  * No scatter phase: the host packs the RAW pixels in scan-slot order
    (xT2 col = 64d + 16bi + s, bwd copy right-aligned within each 16-slot
    block), so the in_proj psum evictions write X/Xr directly with
    dst-contiguous 64-col runs. X layout col = 128d + 16bi + s leaves a
    dead upper half per diagonal; windows are offset 257s (fwd) and
    255s+15 (bwd) with strides [[128, nd], [16, 4]], nd = 94-4s.
  * un-scatters moved to the idle GPSIMD engine (3 pieces per dir-step).
  * out_proj: evictions go to per-m contiguous 4096-col slices (ACT for
    even m on dead X, DVE for odd m on dead Xr) and each m DMAs out as
    128 contiguous 8KB descriptors right after its eviction (the V2
    layout produced 4KB descriptors and a ~43 us DMA tail).

Per-core SBUF: X/Xr [128, 12288] bf16, hstall [128, 34, 384] bf16,
C [128, 2, 376] f32, un [128, 8192] bf16, xT2 [3, 12288] bf16,
weights [128, 3584] bf16.
"""
from contextlib import ExitStack

import numpy as np

import concourse.bass as bass
import concourse.tile as tile
from concourse.tile import add_dep_helper
from concourse import mybir
from concourse.bass_utils import run_bass_kernel_spmd

AF = mybir.ActivationFunctionType
F32 = mybir.dt.float32
BF16 = mybir.dt.bfloat16
U32 = mybir.dt.uint32

BS = 4            # batch shard per core
NCORES = 8
H = W = 32
HC = 128
D = 94            # anti-diagonals of the skewed grid
S = 16            # max real-segment length
NCOLX = 96 * 128  # X/Xr alloc: 94 d-blocks (live) padded to 96


def _geom():
    d = np.arange(D)
    r0 = np.maximum(0, d - 62)
    r1 = np.minimum(31, d)
    a = np.maximum(r0, -(-(d - 31) // 2))
    b = np.minimum(r1, d // 2)
    L = b - a + 1
    return a, b, L, a - r0, r1 - b


def _ap(t, off, dims):
    """Free-dim-strided AP on SBUF tile t: dims = [[stride, count], ...]."""
    a = t[:, :]
    return bass.AP(tensor=a.tensor, offset=a.offset + off, ap=[a.ap[0]] + dims)


def build(nc):
    # ---------------- DRAM I/O ----------------
    xT2_d = nc.dram_tensor("xT2", (3, 2 * 6144), BF16, kind="ExternalInput")
    ipw_d = nc.dram_tensor("in_projT", (3, HC), BF16, kind="ExternalInput")
    wpack_d = nc.dram_tensor("wpack", (HC, 3584), BF16, kind="ExternalInput")
    vpack_d = nc.dram_tensor("vpack", (HC, 15), F32, kind="ExternalInput")
    hinit_d = nc.dram_tensor("hinit", (HC, 2 * 376), BF16, kind="ExternalInput")
    cinit_d = nc.dram_tensor("cinit", (HC, 2 * 376), F32, kind="ExternalInput")
    out_d = nc.dram_tensor("out", (HC, 6, BS * 1024), BF16,
                           kind="ExternalOutput")

    with tile.TileContext(nc) as tc, ExitStack() as ctx:
        const = ctx.enter_context(tc.tile_pool(name="const", bufs=1))
        big = ctx.enter_context(tc.tile_pool(name="big", bufs=1))
        etmp = ctx.enter_context(tc.tile_pool(name="etmp", bufs=4))
        psum = ctx.enter_context(tc.tile_pool(name="psum", bufs=1,
                                              space="PSUM"))

        final_insts = []
        ipw = const.tile([3, HC], BF16)
        final_insts.append(nc.sync.dma_start(ipw, ipw_d.ap()))
        vpk = const.tile([HC, 15], F32)
        final_insts.append(nc.sync.dma_start(vpk, vpack_d.ap()))
        xT2 = const.tile([3, 2 * 6144], BF16)
        final_insts.append(nc.sync.dma_start(xT2[:, 0:6144],
                                             xT2_d.ap()[:, 0:6144]))
        final_insts.append(nc.sync.dma_start(xT2[:, 6144:12288],
                                             xT2_d.ap()[:, 6144:12288]))
        # hstall: slice 0/1 = fwd/bwd init h; slice 2+2s+di = step-(s,dir) h
        hstall = const.tile([HC, 34, 384], BF16, name="hstall")
        final_insts.append(nc.sync.dma_start(
            _ap(hstall, 0, [[384, 2], [1, 376]]), hinit_d.ap()))
        C = const.tile([HC, 2, 376], F32, name="C")
        final_insts.append(nc.sync.dma_start(
            _ap(C, 0, [[376, 2], [1, 376]]), cinit_d.ap()))
        wpk = const.tile([HC, 3584], BF16)
        final_insts.append(nc.sync.dma_start(wpk, wpack_d.ap()))

        wi = {'f': wpk[:, 0:512], 'b': wpk[:, 1024:1536]}
        wh = {'f': wpk[:, 512:1024], 'b': wpk[:, 1536:2048]}
        owf = wpk[:, 2048:2816]
        owb = wpk[:, 2816:3584]
        bias = {'f': vpk[:, 0:4], 'b': vpk[:, 4:8]}      # per-gate biases
        ob = vpk[:, 8:14]
        ipb = vpk[:, 14:15]

        X = big.tile([HC, NCOLX], BF16, tag="X")
        Xr = big.tile([HC, NCOLX], BF16, tag="Xr")
        un = big.tile([HC, 2 * BS * 1024], BF16, tag="un")

        # ---- per-engine pre-observers: consume DMA semaphores via tiny
        # single-wait garbage ops so real instructions keep <=1 sync wait.
        trash_a = const.tile([HC, 4], F32)
        trash_d = const.tile([HC, 4], F32)
        nc.tensor.ldweights(ipw[:, 0:1])                 # ipw DMA
        nc.tensor.ldweights(xT2[:, 4095:4096])           # xT2 first-half DMA
        nc.tensor.ldweights(xT2[:, 12287:12288])         # xT2 second-half DMA
        nc.scalar.activation(trash_a[:, 0:1], vpk[:, 0:1], AF.Copy)   # vpack
        nc.scalar.activation(trash_a[:, 1:2], C[:, 0, 0:1], AF.Copy)  # cinit
        nc.vector.tensor_copy(trash_d[:, 0:1], vpk[:, 0:1])           # vpack
        nc.vector.tensor_copy(trash_d[:, 1:2], C[:, 0, 0:1])          # cinit
        tc.no_sync_barrier()

        # ---------------- phase 1: in_proj -> X / Xr ----------------
        # 24 psum chunks of 512 (12 per buffer); tile q covers chunks
        # 4q..4q+3; X tiles evicted by ACT, Xr tiles by DVE, interleaved so
        # both engines run while the PE streams ahead.
        with nc.named_scope("in_proj"):
            last_x_op = last_xr_op = None
            for q in range(6):
                half = q % 2               # 0 -> X (ACT), 1 -> Xr (DVE)
                ti = q // 2                # tile index within the buffer
                P = psum.tile([HC, 4, 512], F32, tag='P' + 'fb'[half],
                              name=f"pin{q}")
                for cb in range(4):
                    ci = half * 12 + ti * 4 + cb
                    nc.tensor.matmul(P[:, cb, :], ipw,
                                     xT2[:, 512 * ci:512 * ci + 512],
                                     start=True, stop=True)
                for cb in range(4):
                    dchunk = (ti * 4 + cb) * 8       # first diag of chunk
                    dst = _ap(X if half == 0 else Xr, 128 * dchunk,
                              [[128, 8], [1, 64]])
                    if half == 0:
                        last_x_op = nc.scalar.activation(
                            dst, P[:, cb, :], AF.Identity, bias=ipb)
                    else:
                        last_xr_op = nc.vector.tensor_scalar_add(
                            dst, P[:, cb, :], ipb)
            # late consts: observed after in_proj mms are queued so the PE
            # can start projecting while these DMAs are still in flight.
            nc.tensor.ldweights(wpk[:, 0:1])                  # wpack DMA
            nc.tensor.ldweights(hstall[:, 0, 0:1])            # hinit DMA

        a_g, b_g, L_g, jf_g, jb_g = _geom()

        # ---------------- phase 2: scan (+ fused un scatter) ------------
        Xbuf = {'f': X, 'b': Xr}
        with nc.named_scope("scan"):
            # PE pre-observes eviction completion (frees pin psum tags AND
            # guarantees X/Xr contents) via two garbage ldweights.
            # cells written by the LAST eviction chunk (dchunk 88, col 11264)
            ldx = nc.tensor.ldweights(X[:, 11264:11265])      # ACT tick
            ldxr = nc.tensor.ldweights(Xr[:, 11264:11265])    # DVE tick
            first_mm_deps = [ldx, ldxr]
            sfio_prev = {}
            hst_prev = {'f': hstall[:, 0, :], 'b': hstall[:, 1, :]}
            tc_prev = {}
            for s in range(S):
                nd = 94 - 4 * s
                N = BS * nd
                step = {}
                for di, dr in enumerate('fb'):
                    P = psum.tile([HC, 4, 512], F32, tag='P' + dr,
                                  name=f"P{dr}{s}")
                    off = 257 * s if dr == 'f' else 255 * s + 15
                    xap = _ap(Xbuf[dr], off, [[128, nd], [16, BS]])
                    if s >= 1:
                        # PE pre-observes the ACT tick that freed this dir's
                        # psum banks (sigma_o of s-1 is the last psum reader)
                        ldw = nc.tensor.ldweights(sfio_prev[dr][0:1, 2, 0:1])
                        deps0 = [ldw]
                    else:
                        deps0 = first_mm_deps
                    for g in range(4):
                        mmi = nc.tensor.matmul(
                            P[:, g, 0:N], wi[dr][:, g * HC:(g + 1) * HC],
                            xap, start=True, stop=False)
                        if g == 0:
                            for dep in deps0:
                                add_dep_helper(mmi.ins, dep.ins, sync=False,
                                               reason="pre-observed")
                    h0 = 0 if s == 0 else 8
                    hap = hst_prev[dr][:, h0:h0 + N]
                    for g in range(4):
                        nc.tensor.matmul(P[:, g, 0:N],
                                         wh[dr][:, g * HC:(g + 1) * HC],
                                         hap, start=False, stop=True)
                    # ---- activations in CONSUMER order: cm needs sigma_f
                    # first, pm needs sigma_i + tg, sigma_o last ----
                    sfio = etmp.tile([HC, 3, 384], BF16, tag="sfio",
                                     name=f"sfio{dr}{s}")
                    sfio_prev[dr] = sfio
                    tgt = etmp.tile([HC, 384], BF16, tag="tg",
                                    name=f"tg{dr}{s}")
                    for g in (1, 0, 3, 2):
                        if g == 3:
                            nc.scalar.activation(tgt[:, 0:N], P[:, 3, 0:N],
                                                 AF.Tanh,
                                                 bias=bias[dr][:, 3:4])
                        else:
                            nc.scalar.activation(sfio[:, g, 0:N], P[:, g, 0:N],
                                                 AF.Sigmoid,
                                                 bias=bias[dr][:, g:g + 1])
                    # ---- cell update: c = sf*c + si*tg (c preloaded by the
                    # cinit DMA at s=0, shifted window afterwards) ----
                    c_sl = C[:, di, 8 * s:8 * s + N]
                    cm = nc.vector.tensor_mul(c_sl, c_sl, sfio[:, 1, 0:N])
                    nc.vector.tensor_mul(tgt[:, 0:N], sfio[:, 0, 0:N],
                                         tgt[:, 0:N])
                    nc.vector.tensor_add(c_sl, c_sl, tgt[:, 0:N])
                    step[dr] = (sfio, c_sl)
                for di, dr in enumerate('fb'):
                    sfio, c_sl = step[dr]
                    tct = etmp.tile([HC, 384], BF16, tag="tc",
                                    name=f"tc{dr}{s}")
                    nc.scalar.activation(tct[:, 0:N], c_sl, AF.Tanh)
                    tc_prev[dr] = tct
                    hst = hstall[:, 2 + 2 * s + di, :]
                    nc.vector.tensor_mul(hst[:, 0:N], sfio[:, 2, 0:N],
                                         tct[:, 0:N])
                    hst_prev[dr] = hst
                    # scatter this step's h into un on the idle GPSIMD:
                    # pieces = (src_off, src_stride, cnt, dst_off, dst_stride)
                    if dr == 'f':
                        pieces = [(0, 4, 32 - 2 * s, 32 * s, 1),
                                  (4 * (32 - 2 * s), 8, 31 - s,
                                   30 * s + 62, 32),
                                  (4 * (33 - 2 * s), 8, 31 - s,
                                   30 * s + 63, 32)]
                    else:
                        pieces = [(0, 8, 32 - s, 4096 + 2 * s, 32),
                                  (4, 8, 31 - s, 4096 + 2 * s + 1, 32),
                                  (4 * (63 - 2 * s), 4, 31 - 2 * s,
                                   4096 + 993 - 30 * s, 1)]
                    for soff, sstr, cnt, doff, dstr in pieces:
                        hsrc = bass.AP(
                            tensor=hst.tensor, offset=hst.offset + soff,
                            ap=[hst.ap[0]] + [[1, BS], [sstr, cnt]])
                        last_un = nc.gpsimd.tensor_copy(
                            _ap(un, doff, [[1024, BS], [dstr, cnt]]), hsrc)

        # ---------------- phase 3: output projection ----------------
        # m-th output chunk evicts (with bias) to a contiguous 4096-col
        # slice of dead X (ACT, even m) or dead Xr (DVE, odd m), then DMAs
        # out as 128 contiguous 8KB descriptors.
        with nc.named_scope("out_proj"):
            ldun = nc.tensor.ldweights(un[:, 4639:4640])            # POOL tick
            ldtg = nc.tensor.ldweights(tc_prev['b'][:, 0:1])        # ACT tick
            evA = big.tile([HC, NCOLX], BF16, tag="X", name="evA")
            evB = big.tile([HC, NCOLX], BF16, tag="Xr", name="evB")

            last_ev = {0: None, 1: None}   # per-tag last evict dst
            for m in range(6):
                ev = evA if m % 2 == 0 else evB
                ow_m_f = owf[:, m * HC:(m + 1) * HC]
                ow_m_b = owb[:, m * HC:(m + 1) * HC]
                for half in range(2):
                    P = psum.tile([HC, 4, 512], F32, tag='P' + 'fb'[half],
                                  name=f"Po{m}{half}")
                    deps = []
                    if m == 0 and half == 0:
                        deps = [ldun, ldtg]
                    elif m == 0 and half == 1:
                        deps = [nc.tensor.ldweights(tc_prev['b'][:, 0:1])]
                    elif last_ev[half] is not None:
                        # PE pre-observes the evict tick that freed this tag
                        deps.append(nc.tensor.ldweights(last_ev[half]))
                        last_ev[half] = None
                    for cb in range(4):
                        ch = half * 4 + cb
                        mmi = nc.tensor.matmul(
                            P[:, cb, :], ow_m_f,
                            un[:, ch * 512:(ch + 1) * 512],
                            start=True, stop=False)
                        if cb == 0:
                            for dep in deps:
                                add_dep_helper(mmi.ins, dep.ins, sync=False,
                                               reason="pre-observed")
                        last_mm = nc.tensor.matmul(
                            P[:, cb, :], ow_m_b,
                            un[:, 4096 + ch * 512:4096 + (ch + 1) * 512],
                            start=False, stop=True)
                    for cb in range(4):
                        ch = half * 4 + cb
                        dst = ev[:, (m // 2) * 4096 + ch * 512:
                                 (m // 2) * 4096 + (ch + 1) * 512]
                        if m % 2 == 0:
                            last_act_evi = nc.scalar.activation(
                                dst, P[:, cb, :], AF.Identity,
                                bias=ob[:, m:m + 1])
                            last_ev[half] = dst[:, 0:1]
                        else:
                            last_dve_evi = nc.vector.tensor_scalar_add(
                                dst, P[:, cb, :], ob[:, m:m + 1])
                            last_ev[half] = dst[:, 0:1]
                # per-m DMA: src/dst contiguous per partition (8KB descs)
                final_insts.append(nc.gpsimd.dma_start(
                    out_d.ap()[:, m, :],
                    ev[:, (m // 2) * 4096:(m // 2) * 4096 + 4096]))
            final_insts += [last_mm, last_act_evi, last_dve_evi, last_un]
            for fi in final_insts:
                nop = nc.sync.nop()
                add_dep_helper(nop.ins, fi.ins, sync=True,
                               reason="drain diet: pre-observe final ticks")
    return nc


def _lstm_pad_states(Wh, b):
    """State after j pad steps (x=0): gates = b + Wh@h. Returns (17,HC) x2."""
    h = np.zeros(HC, np.float32)
    c = np.zeros(HC, np.float32)
    hs, cs = [h], [c]
    for _ in range(S):
        g = b + h @ Wh.T
        i, f, o, gg = g[0:HC], g[HC:2 * HC], g[2 * HC:3 * HC], g[3 * HC:]
        sig = lambda z: 1.0 / (1.0 + np.exp(-z))
        c = sig(f) * c + sig(i) * np.tanh(gg)
        h = sig(o) * np.tanh(c)
        hs.append(h.astype(np.float32))
        cs.append(c.astype(np.float32))
    return np.stack(hs), np.stack(cs)


def _pack_indices():
    """Host gather indices: xT2 col -> (bi, r, w) for live slots."""
    a, b, L, jf, jb = _geom()
    cols_f, cols_b = [], []
    src_b, src_r, src_w = [], [], []
    for d in range(D):
        for bi in range(BS):
            for s in range(int(L[d])):
                r = int(a[d]) + s
                w = d - 2 * r
                cols_f.append(64 * d + 16 * bi + s)
                cols_b.append(64 * d + 16 * bi + 16 - int(L[d]) + s)
                src_b.append(bi)
                src_r.append(r)
                src_w.append(w)
    return (np.array(cols_f), np.array(cols_b), np.array(src_b),
            np.array(src_r), np.array(src_w))


_PACK = _pack_indices()


def _prep_inputs(inputs):
    """Host-side weight reshaping + pixel packing -> per-core in_maps."""
    import ml_dtypes
    bf = ml_dtypes.bfloat16

    def cast(a):
        return np.ascontiguousarray(a, np.float32).astype(bf)

    x = np.asarray(inputs['x'], np.float32)
    fwd_Wh = np.asarray(inputs['fwd_Wh'], np.float32)
    bwd_Wh = np.asarray(inputs['bwd_Wh'], np.float32)
    fwd_b = np.asarray(inputs['fwd_b'], np.float32)
    bwd_b = np.asarray(inputs['bwd_b'], np.float32)
    wpack = np.concatenate([
        np.asarray(inputs['fwd_Wi'], np.float32).T, fwd_Wh.T,
        np.asarray(inputs['bwd_Wi'], np.float32).T, bwd_Wh.T,
        np.asarray(inputs['out_w'], np.float32)[:, :HC].T,
        np.asarray(inputs['out_w'], np.float32)[:, HC:].T,
    ], axis=1)                                             # (128, 3584)
    vpack = np.concatenate([
        fwd_b.reshape(4, HC).T, bwd_b.reshape(4, HC).T,
        np.asarray(inputs['out_b'], np.float32).reshape(6, HC).T,
        np.asarray(inputs['in_proj_b'], np.float32).reshape(HC, 1),
    ], axis=1)                                             # (128, 15)

    # per-diagonal init states from the pad-state tables
    a, b, L, jf, jb = _geom()
    hinit = np.zeros((HC, 2 * 376), np.float32)
    cinit = np.zeros((HC, 2 * 376), np.float32)
    for di, (Wh_, b_) in enumerate(((fwd_Wh, fwd_b), (bwd_Wh, bwd_b))):
        hs, cs = _lstm_pad_states(Wh_, b_)
        j = jf if di == 0 else jb
        cols = 376 * di + 4 * np.arange(D)[:, None] + np.arange(BS)[None, :]
        hinit[:, cols.reshape(-1)] = np.repeat(hs[j], BS, axis=0).T
        cinit[:, cols.reshape(-1)] = np.repeat(cs[j], BS, axis=0).T

    common = {
        "in_projT": cast(np.asarray(inputs['in_proj_w'], np.float32).T
                         / 255.0),
        "wpack": cast(wpack),
        "vpack": np.ascontiguousarray(vpack),
        "hinit": cast(hinit),
        "cinit": np.ascontiguousarray(cinit),
    }
    cols_f, cols_b, sb, sr, sw = _PACK
    in_maps = []
    for c in range(NCORES):
        xs = x[c * BS:(c + 1) * BS]                        # (4, 3, 32, 32)
        vals = xs[sb, :, sr, sw].T                         # (3, nlive)
        xT2c = np.zeros((3, 2 * 6144), np.float32)
        xT2c[:, cols_f] = vals
        xT2c[:, 6144 + cols_b] = vals
        in_maps.append({"xT2": cast(xT2c), **common})
    return in_maps


def _assemble(results):
    outs = []
    for r in results:
        lg = np.asarray(r["out"], dtype=np.float32)        # (128, 6, 4096)
        lg = lg.transpose(1, 0, 2).reshape(6, HC, BS, H, W)
        lg = lg.transpose(2, 0, 1, 3, 4)
        outs.append(lg.reshape(BS, 768, H, W))
    full = np.concatenate(outs, axis=0)
    return np.ascontiguousarray(
        full.reshape(32, 3, 256, H, W).astype(np.float32))


def kernel(**inputs):
    nc = bass.Bass("TRN2", target_bir_lowering=False, debug=False)
    build(nc)
    in_maps = _prep_inputs(inputs)
    res = run_bass_kernel_spmd(nc, in_maps, core_ids=list(range(NCORES)))
    return _assemble(res.results)


if __name__ == "__main__":
    nc = bass.Bass("TRN2", target_bir_lowering=False, debug=False)
    build(nc)
    print("IR build OK")


# revision 21
# speedup vs baseline: 1.6694x; 1.0634x over previous
"""Trainium2 Bass kernel V3 for the PixelRNN Diagonal BiLSTM problem.

Contract: kernel(**inputs) takes FULL unsharded inputs and returns the FULL
(32, 3, 256, 32, 32) float32 output. Pure data-parallel over 8 NeuronCores
(4 images each), weights replicated, no collectives.

V3 vs V2 (see sim_v3.py for geometry validation):
  * Halved scan: only ~half the skewed-grid cells are real pixels. Prefix
    pad-cell LSTM states are input-independent (x=0 there), so the host
    precomputes the 17 possible pad states from the weights and each
    diagonal's scan starts at its first real cell with a per-diagonal init
    (h_init, c_init) table. Real segments are <= 16 steps, so the scan is
    16 macro-steps/dir over 4096 columns (vs 32 steps over 8064).
  * No scatter phase: the host packs the RAW pixels in scan-slot order
    (xT2 col = 64d + 16bi + s, bwd copy right-aligned within each 16-slot
    block), so the in_proj psum evictions write X/Xr directly with
    dst-contiguous 64-col runs. X layout col = 128d + 16bi + s leaves a
    dead upper half per diagonal; windows are offset 257s (fwd) and
    255s+15 (bwd) with strides [[128, nd], [16, 4]], nd = 94-4s.
  * un-scatters moved to the idle GPSIMD engine (3 pieces per dir-step).
  * out_proj: evictions go to per-m contiguous 4096-col slices (ACT for
    even m on dead X, DVE for odd m on dead Xr) and each m DMAs out as
    128 contiguous 8KB descriptors right after its eviction (the V2
    layout produced 4KB descriptors and a ~43 us DMA tail).

Per-core SBUF: X/Xr [128, 12288] bf16, hstall [128, 34, 384] bf16,
C [128, 2, 376] f32, un [128, 8192] bf16, xT2 [3, 12288] bf16,
weights [128, 3584] bf16.
"""
from contextlib import ExitStack

import numpy as np

import concourse.bass as bass
import concourse.tile as tile
from concourse.tile import add_dep_helper
from concourse import mybir
from concourse.bass_utils import run_bass_kernel_spmd

AF = mybir.ActivationFunctionType
F32 = mybir.dt.float32
BF16 = mybir.dt.bfloat16
U32 = mybir.dt.uint32

BS = 4            # batch shard per core
NCORES = 8
H = W = 32
HC = 128
D = 94            # anti-diagonals of the skewed grid
S = 16            # max real-segment length
NCOLX = 96 * 128  # X/Xr alloc: 94 d-blocks (live) padded to 96


def _geom():
    d = np.arange(D)
    r0 = np.maximum(0, d - 62)
    r1 = np.minimum(31, d)
    a = np.maximum(r0, -(-(d - 31) // 2))
    b = np.minimum(r1, d // 2)
    L = b - a + 1
    return a, b, L, a - r0, r1 - b


def _ap(t, off, dims):
    """Free-dim-strided AP on SBUF tile t: dims = [[stride, count], ...]."""
    a = t[:, :]
    return bass.AP(tensor=a.tensor, offset=a.offset + off, ap=[a.ap[0]] + dims)


def build(nc):
    # ---------------- DRAM I/O ----------------
    xT2_d = nc.dram_tensor("xT2", (3, 2 * 6144), BF16, kind="ExternalInput")
    ipw_d = nc.dram_tensor("in_projT", (3, HC), BF16, kind="ExternalInput")
    wpack_d = nc.dram_tensor("wpack", (HC, 3584), BF16, kind="ExternalInput")
    vpack_d = nc.dram_tensor("vpack", (HC, 15), F32, kind="ExternalInput")
    hinit_d = nc.dram_tensor("hinit", (HC, 2 * 376), BF16, kind="ExternalInput")
    cinit_d = nc.dram_tensor("cinit", (HC, 2 * 376), F32, kind="ExternalInput")
    out_d = nc.dram_tensor("out", (HC, 6, BS * 1024), BF16,
                           kind="ExternalOutput")

    with tile.TileContext(nc) as tc, ExitStack() as ctx:
        const = ctx.enter_context(tc.tile_pool(name="const", bufs=1))
        big = ctx.enter_context(tc.tile_pool(name="big", bufs=1))
        etmp = ctx.enter_context(tc.tile_pool(name="etmp", bufs=4))
        psum = ctx.enter_context(tc.tile_pool(name="psum", bufs=1,
                                              space="PSUM"))

        # DMA order: in_proj deps first (ipw, xT2 in 4 slices, vpk), then
        # scan deps (cinit, hinit, wpack) which overlap in_proj compute.
        final_insts = []
        ipw = const.tile([3, HC], BF16)
        final_insts.append(nc.sync.dma_start(ipw, ipw_d.ap()))
        xT2 = const.tile([3, 2 * 6144], BF16)
        xdmas = []
        for lo, hi in ((0, 3072), (6144, 9216), (3072, 6144), (9216, 12288)):
            xdmas.append(nc.sync.dma_start(xT2[:, lo:hi],
                                           xT2_d.ap()[:, lo:hi]))
        final_insts += xdmas
        vpk = const.tile([HC, 15], F32)
        final_insts.append(nc.sync.dma_start(vpk, vpack_d.ap()))
        cini = const.tile([HC, 2, 384], F32, name="cini")
        final_insts.append(nc.sync.dma_start(
            _ap(cini, 0, [[384, 2], [1, 376]]), cinit_d.ap()))
        C = const.tile([HC, 2, 384], F32, name="C")
        # hstall: slice 0/1 = fwd/bwd init h; slice 2+2s+di = step-(s,dir) h
        hstall = const.tile([HC, 34, 384], BF16, name="hstall")
        final_insts.append(nc.sync.dma_start(
            _ap(hstall, 0, [[384, 2], [1, 376]]), hinit_d.ap()))
        wpk = const.tile([HC, 3584], BF16)
        final_insts.append(nc.sync.dma_start(wpk, wpack_d.ap()))

        wi = {'f': wpk[:, 0:512], 'b': wpk[:, 1024:1536]}
        wh = {'f': wpk[:, 512:1024], 'b': wpk[:, 1536:2048]}
        owf = wpk[:, 2048:2816]
        owb = wpk[:, 2816:3584]
        bias = {'f': vpk[:, 0:4], 'b': vpk[:, 4:8]}      # per-gate biases
        ob = vpk[:, 8:14]
        ipb = vpk[:, 14:15]

        X = big.tile([HC, NCOLX], BF16, tag="X")
        Xr = big.tile([HC, NCOLX], BF16, tag="Xr")
        un = big.tile([HC, 2 * BS * 1024], BF16, tag="un")

        # ---- per-engine pre-observers: consume DMA semaphores via tiny
        # single-wait garbage ops so real instructions keep <=1 sync wait.
        trash_a = const.tile([HC, 4], F32)
        trash_d = const.tile([HC, 4], F32)
        nc.tensor.ldweights(ipw[:, 0:1])                 # ipw DMA
        for lo, hi in ((0, 3072), (6144, 9216), (3072, 6144), (9216, 12288)):
            nc.tensor.ldweights(xT2[:, hi - 1:hi])       # xT2 slice DMAs
        nc.scalar.activation(trash_a[:, 0:1], vpk[:, 0:1], AF.Copy)   # vpack
        nc.vector.tensor_copy(trash_d[:, 0:1], vpk[:, 0:1])           # vpack
        # C's sole writer chain starts with this DVE copy of the cinit DMA,
        # so scan cell ops keep single-sem waits (no DMA dep on C). tch
        # carries the DVE self-wait; the s=0 cm ops take a nosync edge on it
        # (V2's "c touch first" idiom).
        nc.vector.tensor_copy(C[:, :, :], cini[:, :, :])
        # col 376+ is copy-written but never cm-written: no WAR back-edge
        tch = nc.vector.tensor_copy(trash_d[:, 1:2], C[:, 0, 376:377])
        tc.no_sync_barrier()

        # ---------------- phase 1: in_proj -> X / Xr ----------------
        # 24 psum chunks of 512 (12 per buffer); tile q covers chunks
        # 4q..4q+3; X tiles evicted by ACT, Xr tiles by DVE, interleaved so
        # both engines run while the PE streams ahead.
        with nc.named_scope("in_proj"):
            last_x_op = last_xr_op = None
            ev_cell = {}                   # half -> last evict dst cell
            for q in range(6):
                half = q % 2               # 0 -> X (ACT), 1 -> Xr (DVE)
                ti = q // 2                # tile index within the buffer
                P = psum.tile([HC, 4, 512], F32, tag='P' + 'fb'[half],
                              name=f"pin{q}")
                deps0 = []
                if ti >= 1:
                    # PE pre-observes the evict tick that freed this tag
                    deps0 = [nc.tensor.ldweights(ev_cell[half])]
                for cb in range(4):
                    ci = half * 12 + ti * 4 + cb
                    mmi = nc.tensor.matmul(P[:, cb, :], ipw,
                                           xT2[:, 512 * ci:512 * ci + 512],
                                           start=True, stop=True)
                    if cb == 0:
                        for dep in deps0:
                            add_dep_helper(mmi.ins, dep.ins, sync=False,
                                           reason="pre-observed")
                # single 2048-col eviction per tile (32 diag blocks)
                dst = _ap(X if half == 0 else Xr, 128 * 32 * ti,
                          [[128, 32], [1, 64]])
                src = P[:, :, :].rearrange("p a b -> p (a b)")
                if half == 0:
                    last_x_op = nc.scalar.activation(
                        dst, src, AF.Identity, bias=ipb)
                else:
                    last_xr_op = nc.vector.tensor_scalar_add(dst, src, ipb)
                ev_cell[half] = (X if half == 0 else Xr)[
                    :, 4096 * ti:4096 * ti + 1]
            # late consts: observed after in_proj mms are queued so the PE
            # can start projecting while these DMAs are still in flight.
            nc.tensor.ldweights(wpk[:, 0:1])                  # wpack DMA
            nc.tensor.ldweights(hstall[:, 0, 0:1])            # hinit DMA

        a_g, b_g, L_g, jf_g, jb_g = _geom()

        # ---------------- phase 2: scan (+ fused un scatter) ------------
        Xbuf = {'f': X, 'b': Xr}
        with nc.named_scope("scan"):
            # PE pre-observes eviction completion (frees pin psum tags AND
            # guarantees X/Xr contents) via two garbage ldweights.
            # cells written by the LAST eviction op (tile ti=2, col 8192)
            ldx = nc.tensor.ldweights(X[:, 8192:8193])        # ACT tick
            ldxr = nc.tensor.ldweights(Xr[:, 8192:8193])      # DVE tick
            first_mm_deps = [ldx, ldxr]
            sfio_prev = {}
            hst_prev = {'f': hstall[:, 0, :], 'b': hstall[:, 1, :]}
            tc_prev = {}
            for s in range(S):
                nd = 94 - 4 * s
                N = BS * nd
                step = {}
                for di, dr in enumerate('fb'):
                    P = psum.tile([HC, 4, 512], F32, tag='P' + dr,
                                  name=f"P{dr}{s}")
                    off = 257 * s if dr == 'f' else 255 * s + 15
                    xap = _ap(Xbuf[dr], off, [[128, nd], [16, BS]])
                    if s >= 1:
                        # PE pre-observes the ACT tick that freed this dir's
                        # psum banks (sigma_o of s-1 is the last psum reader)
                        ldw = nc.tensor.ldweights(sfio_prev[dr][0:1, 2, 0:1])
                        deps0 = [ldw]
                    else:
                        deps0 = first_mm_deps
                    for g in range(4):
                        mmi = nc.tensor.matmul(
                            P[:, g, 0:N], wi[dr][:, g * HC:(g + 1) * HC],
                            xap, start=True, stop=False)
                        if g == 0:
                            for dep in deps0:
                                add_dep_helper(mmi.ins, dep.ins, sync=False,
                                               reason="pre-observed")
                    h0 = 0 if s == 0 else 8
                    hap = hst_prev[dr][:, h0:h0 + N]
                    for g in range(4):
                        nc.tensor.matmul(P[:, g, 0:N],
                                         wh[dr][:, g * HC:(g + 1) * HC],
                                         hap, start=False, stop=True)
                    # ---- activations in CONSUMER order: cm needs sigma_f
                    # first, pm needs sigma_i + tg, sigma_o last ----
                    sfio = etmp.tile([HC, 3, 384], BF16, tag="sfio",
                                     name=f"sfio{dr}{s}")
                    sfio_prev[dr] = sfio
                    tgt = etmp.tile([HC, 384], BF16, tag="tg",
                                    name=f"tg{dr}{s}")
                    for g in (1, 0, 3, 2):
                        if g == 3:
                            nc.scalar.activation(tgt[:, 0:N], P[:, 3, 0:N],
                                                 AF.Tanh,
                                                 bias=bias[dr][:, 3:4])
                        else:
                            nc.scalar.activation(sfio[:, g, 0:N], P[:, g, 0:N],
                                                 AF.Sigmoid,
                                                 bias=bias[dr][:, g:g + 1])
                    # ---- cell update: c = sf*c + si*tg (c preloaded by the
                    # cinit DMA at s=0, shifted window afterwards) ----
                    c_sl = C[:, di, 8 * s:8 * s + N]
                    cm = nc.vector.tensor_mul(c_sl, c_sl, sfio[:, 1, 0:N])
                    if s == 0:
                        add_dep_helper(cm.ins, tch.ins, sync=False,
                                       reason="c touch first")
                    nc.vector.tensor_mul(tgt[:, 0:N], sfio[:, 0, 0:N],
                                         tgt[:, 0:N])
                    nc.vector.tensor_add(c_sl, c_sl, tgt[:, 0:N])
                    step[dr] = (sfio, c_sl)
                for di, dr in enumerate('fb'):
                    sfio, c_sl = step[dr]
                    tct = etmp.tile([HC, 384], BF16, tag="tc",
                                    name=f"tc{dr}{s}")
                    nc.scalar.activation(tct[:, 0:N], c_sl, AF.Tanh)
                    tc_prev[dr] = tct
                    hst = hstall[:, 2 + 2 * s + di, :]
                    nc.vector.tensor_mul(hst[:, 0:N], sfio[:, 2, 0:N],
                                         tct[:, 0:N])
                    hst_prev[dr] = hst
                    # scatter this step's h into un on the idle GPSIMD:
                    # pieces = (src_off, src_stride, cnt, dst_off, dst_stride)
                    if dr == 'f':
                        pieces = [(0, 4, 32 - 2 * s, 32 * s, 1),
                                  (4 * (32 - 2 * s), 8, 31 - s,
                                   30 * s + 62, 32),
                                  (4 * (33 - 2 * s), 8, 31 - s,
                                   30 * s + 63, 32)]
                    else:
                        pieces = [(0, 8, 32 - s, 4096 + 2 * s, 32),
                                  (4, 8, 31 - s, 4096 + 2 * s + 1, 32),
                                  (4 * (63 - 2 * s), 4, 31 - 2 * s,
                                   4096 + 993 - 30 * s, 1)]
                    for soff, sstr, cnt, doff, dstr in pieces:
                        hsrc = bass.AP(
                            tensor=hst.tensor, offset=hst.offset + soff,
                            ap=[hst.ap[0]] + [[1, BS], [sstr, cnt]])
                        last_un = nc.gpsimd.tensor_copy(
                            _ap(un, doff, [[1024, BS], [dstr, cnt]]), hsrc)

        # ---------------- phase 3: output projection ----------------
        # m-th output chunk evicts (with bias) to a contiguous 4096-col
        # slice of dead X (ACT, even m) or dead Xr (DVE, odd m), then DMAs
        # out as 128 contiguous 8KB descriptors.
        with nc.named_scope("out_proj"):
            ldun = nc.tensor.ldweights(un[:, 4639:4640])            # POOL tick
            ldtg = nc.tensor.ldweights(tc_prev['b'][:, 0:1])        # ACT tick
            evA = big.tile([HC, NCOLX], BF16, tag="X", name="evA")
            evB = big.tile([HC, NCOLX], BF16, tag="Xr", name="evB")

            last_ev = {0: None, 1: None}   # per-tag last evict dst
            for m in range(6):
                ev = evA if m % 2 == 0 else evB
                ow_m_f = owf[:, m * HC:(m + 1) * HC]
                ow_m_b = owb[:, m * HC:(m + 1) * HC]
                for half in range(2):
                    P = psum.tile([HC, 4, 512], F32, tag='P' + 'fb'[half],
                                  name=f"Po{m}{half}")
                    deps = []
                    if m == 0 and half == 0:
                        deps = [ldun, ldtg]
                    elif m == 0 and half == 1:
                        deps = [nc.tensor.ldweights(tc_prev['b'][:, 0:1])]
                    elif last_ev[half] is not None:
                        # PE pre-observes the evict tick that freed this tag
                        deps.append(nc.tensor.ldweights(last_ev[half]))
                        last_ev[half] = None
                    for cb in range(4):
                        ch = half * 4 + cb
                        mmi = nc.tensor.matmul(
                            P[:, cb, :], ow_m_f,
                            un[:, ch * 512:(ch + 1) * 512],
                            start=True, stop=False)
                        if cb == 0:
                            for dep in deps:
                                add_dep_helper(mmi.ins, dep.ins, sync=False,
                                               reason="pre-observed")
                        last_mm = nc.tensor.matmul(
                            P[:, cb, :], ow_m_b,
                            un[:, 4096 + ch * 512:4096 + (ch + 1) * 512],
                            start=False, stop=True)
                    # single 2048-col eviction per psum tile
                    dst = ev[:, (m // 2) * 4096 + half * 2048:
                             (m // 2) * 4096 + half * 2048 + 2048]
                    src = P[:, :, :].rearrange("p a b -> p (a b)")
                    if m % 2 == 0:
                        last_act_evi = nc.scalar.activation(
                            dst, src, AF.Identity, bias=ob[:, m:m + 1])
                    else:
                        last_dve_evi = nc.vector.tensor_scalar_add(
                            dst, src, ob[:, m:m + 1])
                    last_ev[half] = dst[:, 0:1]
                # per-m DMA: src/dst contiguous per partition (8KB descs)
                final_insts.append(nc.gpsimd.dma_start(
                    out_d.ap()[:, m, :],
                    ev[:, (m // 2) * 4096:(m // 2) * 4096 + 4096]))
            final_insts += [last_mm, last_act_evi, last_dve_evi, last_un]
            for fi in final_insts:
                nop = nc.sync.nop()
                add_dep_helper(nop.ins, fi.ins, sync=True,
                               reason="drain diet: pre-observe final ticks")
    return nc


def _lstm_pad_states(Wh, b):
    """State after j pad steps (x=0): gates = b + Wh@h. Returns (17,HC) x2."""
    h = np.zeros(HC, np.float32)
    c = np.zeros(HC, np.float32)
    hs, cs = [h], [c]
    for _ in range(S):
        g = b + h @ Wh.T
        i, f, o, gg = g[0:HC], g[HC:2 * HC], g[2 * HC:3 * HC], g[3 * HC:]
        sig = lambda z: 1.0 / (1.0 + np.exp(-z))
        c = sig(f) * c + sig(i) * np.tanh(gg)
        h = sig(o) * np.tanh(c)
        hs.append(h.astype(np.float32))
        cs.append(c.astype(np.float32))
    return np.stack(hs), np.stack(cs)


def _pack_indices():
    """Host gather indices: xT2 col -> (bi, r, w) for live slots."""
    a, b, L, jf, jb = _geom()
    cols_f, cols_b = [], []
    src_b, src_r, src_w = [], [], []
    for d in range(D):
        for bi in range(BS):
            for s in range(int(L[d])):
                r = int(a[d]) + s
                w = d - 2 * r
                cols_f.append(64 * d + 16 * bi + s)
                cols_b.append(64 * d + 16 * bi + 16 - int(L[d]) + s)
                src_b.append(bi)
                src_r.append(r)
                src_w.append(w)
    return (np.array(cols_f), np.array(cols_b), np.array(src_b),
            np.array(src_r), np.array(src_w))


_PACK = _pack_indices()


def _prep_inputs(inputs):
    """Host-side weight reshaping + pixel packing -> per-core in_maps."""
    import ml_dtypes
    bf = ml_dtypes.bfloat16

    def cast(a):
        return np.ascontiguousarray(a, np.float32).astype(bf)

    x = np.asarray(inputs['x'], np.float32)
    fwd_Wh = np.asarray(inputs['fwd_Wh'], np.float32)
    bwd_Wh = np.asarray(inputs['bwd_Wh'], np.float32)
    fwd_b = np.asarray(inputs['fwd_b'], np.float32)
    bwd_b = np.asarray(inputs['bwd_b'], np.float32)
    wpack = np.concatenate([
        np.asarray(inputs['fwd_Wi'], np.float32).T, fwd_Wh.T,
        np.asarray(inputs['bwd_Wi'], np.float32).T, bwd_Wh.T,
        np.asarray(inputs['out_w'], np.float32)[:, :HC].T,
        np.asarray(inputs['out_w'], np.float32)[:, HC:].T,
    ], axis=1)                                             # (128, 3584)
    vpack = np.concatenate([
        fwd_b.reshape(4, HC).T, bwd_b.reshape(4, HC).T,
        np.asarray(inputs['out_b'], np.float32).reshape(6, HC).T,
        np.asarray(inputs['in_proj_b'], np.float32).reshape(HC, 1),
    ], axis=1)                                             # (128, 15)

    # per-diagonal init states from the pad-state tables
    a, b, L, jf, jb = _geom()
    hinit = np.zeros((HC, 2 * 376), np.float32)
    cinit = np.zeros((HC, 2 * 376), np.float32)
    for di, (Wh_, b_) in enumerate(((fwd_Wh, fwd_b), (bwd_Wh, bwd_b))):
        hs, cs = _lstm_pad_states(Wh_, b_)
        j = jf if di == 0 else jb
        cols = 376 * di + 4 * np.arange(D)[:, None] + np.arange(BS)[None, :]
        hinit[:, cols.reshape(-1)] = np.repeat(hs[j], BS, axis=0).T
        cinit[:, cols.reshape(-1)] = np.repeat(cs[j], BS, axis=0).T

    common = {
        "in_projT": cast(np.asarray(inputs['in_proj_w'], np.float32).T
                         / 255.0),
        "wpack": cast(wpack),
        "vpack": np.ascontiguousarray(vpack),
        "hinit": cast(hinit),
        "cinit": np.ascontiguousarray(cinit),
    }
    cols_f, cols_b, sb, sr, sw = _PACK
    in_maps = []
    for c in range(NCORES):
        xs = x[c * BS:(c + 1) * BS]                        # (4, 3, 32, 32)
        vals = xs[sb, :, sr, sw].T                         # (3, nlive)
        xT2c = np.zeros((3, 2 * 6144), np.float32)
        xT2c[:, cols_f] = vals
        xT2c[:, 6144 + cols_b] = vals
        in_maps.append({"xT2": cast(xT2c), **common})
    return in_maps


def _assemble(results):
    outs = []
    for r in results:
        lg = np.asarray(r["out"], dtype=np.float32)        # (128, 6, 4096)
        lg = lg.transpose(1, 0, 2).reshape(6, HC, BS, H, W)
        lg = lg.transpose(2, 0, 1, 3, 4)
        outs.append(lg.reshape(BS, 768, H, W))
    full = np.concatenate(outs, axis=0)
    return np.ascontiguousarray(
        full.reshape(32, 3, 256, H, W).astype(np.float32))


def kernel(**inputs):
    nc = bass.Bass("TRN2", target_bir_lowering=False, debug=False)
    build(nc)
    in_maps = _prep_inputs(inputs)
    res = run_bass_kernel_spmd(nc, in_maps, core_ids=list(range(NCORES)))
    return _assemble(res.results)


if __name__ == "__main__":
    nc = bass.Bass("TRN2", target_bir_lowering=False, debug=False)
    build(nc)
    print("IR build OK")
